# revision 1
# baseline (speedup 1.0000x reference)
"""DomainCalibratedLoss Trainium2 kernel.

loss = mean_n [ logsumexp_c(x[n,c] + log C[d_n,c]) - (x[n,t_n] + log C[d_n,t_n]) ]

Strategy (8-core SPMD, data-parallel over points):
  - Shard the 1M points across 8 NeuronCores (125k each).
  - Per 128-point tile: PE transposes x to [class, point] layout (PSUM),
    ACT computes exp() while evacuating PSUM->SBUF, PE matmuls the
    exp'd tile against the transposed counts table giving all 3 domain
    sums S[n,d] directly in [point, domain] layout, DVE selects S[n,d_n]
    via a domain one-hot and reduces, ACT takes the log.
    (No max-subtraction needed: inputs are N(0,1) and counts <= 1e4, so
    S <= ~1e9 stays comfortably inside fp32 range.)
  - The per-point target term x[n,t_n] + log C[d_n,t_n] is marshalled on
    the host into a dense [N] side stream ("sub") during sharding, and
    subtracted on-device.  (This platform exposes no per-partition
    gather: tensor_tensor_reduce / scalar_tensor_tensor(accum) /
    tensor_mask_reduce all fault on this runtime, and indirect DMA costs
    ~1.3us per 128 elements.)
  - Device accumulates sum(lnS - sub) per partition; host reduces the
    8x[128,16] partials and divides by the valid count.
"""

import sys

sys.path.insert(0, "/opt/trn_rl_repo")

import numpy as np

import concourse.bass as bass
import concourse.bacc as bacc
import concourse.tile as tile
from concourse import mybir
from concourse.bass_utils import run_bass_kernel_spmd
from concourse.masks import make_identity

P = 128          # partitions / points per tile
C = 200          # classes
D = 3            # domains
C0, C1 = 128, 72 # class chunks (200 = 128 + 72)
W = 2            # point-groups (tiles) per pair
N_CORES = 8

_PROGRAM_CACHE = {}


def build_program(s_per, u_pairs, n_iters, tail, reps=1):
    """Build + compile the SPMD program for one core's shard.

    s_per: points per core; must equal n_iters*u_pairs*W*P + tail.
    reps: outer repetition loop (timing only; reps=1 has no outer loop).
    """
    key = (s_per, u_pairs, n_iters, tail, reps)
    if key in _PROGRAM_CACHE:
        return _PROGRAM_CACHE[key]

    nb_rows = n_iters * u_pairs * W + (1 if tail else 0)
    assert n_iters * u_pairs * W * P + tail == s_per

    nc = bacc.Bacc("TRN2", target_bir_lowering=False, debug=False,
                   num_devices=N_CORES)
    x_in = nc.dram_tensor("x", [s_per, C], mybir.dt.float32,
                          kind="ExternalInput").ap()
    sub_in = nc.dram_tensor("sub_t", [nb_rows, P], mybir.dt.float32,
                            kind="ExternalInput").ap()
    dom_in = nc.dram_tensor("dom_t", [nb_rows, P], mybir.dt.float32,
                            kind="ExternalInput").ap()
    cb_in = nc.dram_tensor("cb", [P, 2 * D], mybir.dt.float32,
                           kind="ExternalInput").ap()
    i33_in = nc.dram_tensor("i33", [P, D], mybir.dt.float32,
                            kind="ExternalInput").ap()
    acc_out = nc.dram_tensor("acc", [P, u_pairs * W], mybir.dt.float32,
                             kind="ExternalOutput").ap()

    with tile.TileContext(nc) as tc:
        with (
            tc.tile_pool(name="singles", bufs=1) as singles,
            tc.tile_pool(name="xp", bufs=3) as xp,
            tc.tile_pool(name="etp", bufs=3) as etp,
            tc.tile_pool(name="tiny", bufs=4) as tiny,
            tc.tile_pool(name="psA", bufs=3, space="PSUM") as psA,
            tc.tile_pool(name="psB", bufs=3, space="PSUM") as psB,
        ):
            ident = singles.tile([P, P], mybir.dt.float32)
            make_identity(nc, ident)
            cb = singles.tile([P, 2 * D], mybir.dt.float32)
            nc.sync.dma_start(out=cb[:], in_=cb_in[:])
            i33 = singles.tile([P, D], mybir.dt.float32)
            nc.sync.dma_start(out=i33[:], in_=i33_in[:])

            acc = singles.tile([P, u_pairs, W], mybir.dt.float32)
            nc.vector.memset(acc[:], 0.0)

            def pair_block(pi, u):
                """One pair = W=2 adjacent 128-point tiles, pi = pair index."""
                xt = xp.tile([P, W, C], mybir.dt.float32, tag="xt")
                nc.sync.dma_start(
                    out=xt[:],
                    in_=x_in[bass.ts(pi, W * P), :].rearrange(
                        "(g p) c -> p g c", p=P),
                )
                sb2 = tiny.tile([P, W], mybir.dt.float32, tag="sb2")
                nc.sync.dma_start(
                    out=sb2[:],
                    in_=sub_in[bass.ts(pi, W), :].rearrange("w p -> p w"),
                )
                dm2 = tiny.tile([P, W], mybir.dt.float32, tag="dm2")
                nc.sync.dma_start(
                    out=dm2[:],
                    in_=dom_in[bass.ts(pi, W), :].rearrange("w p -> p w"),
                )

                pt_ps = psA.tile([P, W, 2, P], mybir.dt.float32, tag="pt")
                for g in range(W):
                    nc.tensor.transpose(out=pt_ps[:, g, 0, :],
                                        in_=xt[:, g, 0:C0],
                                        identity=ident[:])
                    nc.tensor.transpose(out=pt_ps[:C1, g, 1, :],
                                        in_=xt[:, g, C0:C],
                                        identity=ident[:])
                et = etp.tile([P, W, 2, P], mybir.dt.float32, tag="et")
                for g in range(W):
                    nc.scalar.activation(et[:, g, 0, :], pt_ps[:, g, 0, :],
                                         mybir.ActivationFunctionType.Exp)
                    nc.scalar.activation(et[:C1, g, 1, :], pt_ps[:C1, g, 1, :],
                                         mybir.ActivationFunctionType.Exp)
                s36 = psB.tile([P, W, 2, D], mybir.dt.float32, tag="s36")
                for g in range(W):
                    nc.tensor.matmul(s36[:, g, 0, :], lhsT=et[:, g, 0, :],
                                     rhs=cb[:, 0:D], start=True, stop=True)
                    nc.tensor.matmul(s36[:, g, 1, :], lhsT=et[:C1, g, 1, :],
                                     rhs=cb[:C1, D:2 * D], start=True,
                                     stop=True)

                # domain one-hot select + reduce
                tm33 = tiny.tile([P, W, D], mybir.dt.float32, tag="tm33")
                i33b = bass.AP(tensor=i33.tensor, offset=i33.offset,
                               ap=[i33.ap[0], [0, W], i33.ap[1]])
                dmb = bass.AP(tensor=dm2.tensor, offset=dm2.offset,
                              ap=[dm2.ap[0], dm2.ap[1], [0, D]])
                nc.vector.tensor_tensor(out=tm33[:], in0=i33b, in1=dmb,
                                        op=mybir.AluOpType.is_equal)
                m6 = tiny.tile([P, W, 2, D], mybir.dt.float32, tag="m6")
                tm33b = bass.AP(tensor=tm33.tensor, offset=tm33.offset,
                                ap=[tm33.ap[0], tm33.ap[1], [0, 2],
                                    tm33.ap[2]])
                nc.vector.tensor_tensor(out=m6[:], in0=s36[:], in1=tm33b,
                                        op=mybir.AluOpType.mult)
                s2 = tiny.tile([P, W], mybir.dt.float32, tag="s2")
                nc.vector.tensor_reduce(
                    out=s2[:], in_=m6[:].rearrange("p w k d -> p w (k d)"),
                    axis=mybir.AxisListType.X, op=mybir.AluOpType.add)
                lns = tiny.tile([P, W], mybir.dt.float32, tag="lns")
                nc.scalar.activation(lns[:], s2[:],
                                     mybir.ActivationFunctionType.Ln)
                d2 = tiny.tile([P, W], mybir.dt.float32, tag="d2")
                nc.vector.tensor_tensor(out=d2[:], in0=lns[:], in1=sb2[:],
                                        op=mybir.AluOpType.subtract)
                nc.vector.tensor_tensor(out=acc[:, u, :], in0=acc[:, u, :],
                                        in1=d2[:], op=mybir.AluOpType.add)

            def tail_block():
                """Last partial tile of `tail` (<128) points."""
                t = tail
                row0 = n_iters * u_pairs * W * P
                trow = nb_rows - 1
                xt = xp.tile([P, 1, C], mybir.dt.float32, tag="xt")
                nc.sync.dma_start(out=xt[:t, 0, :], in_=x_in[row0:row0 + t, :])
                sb1 = tiny.tile([P, 1], mybir.dt.float32, tag="sb2")
                nc.sync.dma_start(
                    out=sb1[:t, :],
                    in_=sub_in[trow:trow + 1, 0:t].rearrange("w p -> p w"))
                dm1 = tiny.tile([P, 1], mybir.dt.float32, tag="dm2")
                nc.sync.dma_start(
                    out=dm1[:t, :],
                    in_=dom_in[trow:trow + 1, 0:t].rearrange("w p -> p w"))

                pt_ps = psA.tile([P, W, 2, P], mybir.dt.float32, tag="pt")
                nc.tensor.transpose(out=pt_ps[:, 0, 0, 0:t],
                                    in_=xt[:t, 0, 0:C0],
                                    identity=ident[:t, :t])
                nc.tensor.transpose(out=pt_ps[:C1, 0, 1, 0:t],
                                    in_=xt[:t, 0, C0:C],
                                    identity=ident[:t, :t])
                et = etp.tile([P, W, 2, P], mybir.dt.float32, tag="et")
                nc.scalar.activation(et[:, 0, 0, 0:t], pt_ps[:, 0, 0, 0:t],
                                     mybir.ActivationFunctionType.Exp)
                nc.scalar.activation(et[:C1, 0, 1, 0:t], pt_ps[:C1, 0, 1, 0:t],
                                     mybir.ActivationFunctionType.Exp)
                s36 = psB.tile([P, W, 2, D], mybir.dt.float32, tag="s36")
                nc.tensor.matmul(s36[:t, 0, 0, :], lhsT=et[:, 0, 0, 0:t],
                                 rhs=cb[:, 0:D], start=True, stop=True)
                nc.tensor.matmul(s36[:t, 0, 1, :], lhsT=et[:C1, 0, 1, 0:t],
                                 rhs=cb[:C1, D:2 * D], start=True, stop=True)

                tm33 = tiny.tile([P, 1, D], mybir.dt.float32, tag="tm33")
                dmb = bass.AP(tensor=dm1.tensor, offset=dm1.offset,
                              ap=[[dm1.ap[0][0], t], dm1.ap[1], [0, D]])
                i33c = bass.AP(tensor=i33.tensor, offset=i33.offset,
                               ap=[[i33.ap[0][0], t], [0, 1], i33.ap[1]])
                nc.vector.tensor_tensor(out=tm33[:t], in0=i33c, in1=dmb,
                                        op=mybir.AluOpType.is_equal)
                m6 = tiny.tile([P, 1, 2, D], mybir.dt.float32, tag="m6")
                tm33b = bass.AP(tensor=tm33.tensor, offset=tm33.offset,
                                ap=[[tm33.ap[0][0], t], tm33.ap[1], [0, 2],
                                    tm33.ap[2]])
                nc.vector.tensor_tensor(out=m6[:t], in0=s36[:t, 0:1],
                                        in1=tm33b, op=mybir.AluOpType.mult)
                s1 = tiny.tile([P, 1], mybir.dt.float32, tag="s2")
                nc.vector.tensor_reduce(
                    out=s1[:t], in_=m6[:t].rearrange("p w k d -> p w (k d)"),
                    axis=mybir.AxisListType.X, op=mybir.AluOpType.add)
                lns = tiny.tile([P, 1], mybir.dt.float32, tag="lns")
                nc.scalar.activation(lns[:t], s1[:t],
                                     mybir.ActivationFunctionType.Ln)
                d1 = tiny.tile([P, 1], mybir.dt.float32, tag="d2")
                nc.vector.tensor_tensor(out=d1[:t], in0=lns[:t], in1=sb1[:t],
                                        op=mybir.AluOpType.subtract)
                nc.vector.tensor_tensor(out=acc[:t, 0, 0:1],
                                        in0=acc[:t, 0, 0:1], in1=d1[:t],
                                        op=mybir.AluOpType.add)

            def one_pass():
                if n_iters > 0:
                    with tc.For_i(0, n_iters) as it:
                        for u in range(u_pairs):
                            pair_block(it * u_pairs + u, u)
                if tail:
                    tail_block()

            if reps == 1:
                one_pass()
            else:
                with tc.For_i(0, reps):
                    one_pass()

            nc.sync.dma_start(
                out=acc_out[:],
                in_=acc[:].rearrange("p u w -> p (u w)"))

    nc.compile()
    _PROGRAM_CACHE[key] = nc
    return nc


def _host_prep(inputs, targets, domains, domain_counts, s_per, nb_rows):
    """Build the per-core input maps (host-side sharding/marshalling)."""
    n = inputs.shape[0]
    logc = np.log(domain_counts.astype(np.float32))
    tgt = targets.astype(np.int64).reshape(-1)
    dom = domains.astype(np.int64).reshape(-1)
    # dense per-point target term: x[n, t_n] + log C[d_n, t_n]
    sub = inputs[np.arange(n), tgt] + logc[dom, tgt]          # [N] f32

    cb = np.zeros((P, 2 * D), dtype=np.float32)
    ct = np.ascontiguousarray(domain_counts.astype(np.float32).T)  # [C, D]
    cb[:, 0:D] = ct[0:C0]
    cb[0:C1, D:2 * D] = ct[C0:C]
    i33 = np.broadcast_to(np.arange(D, dtype=np.float32), (P, D)).copy()

    in_maps = []
    for c in range(N_CORES):
        lo = c * s_per
        x_c = inputs[lo:lo + s_per]
        sub_c = np.zeros((nb_rows, P), dtype=np.float32)
        dom_c = np.zeros((nb_rows, P), dtype=np.float32)
        full = (s_per // P) * P
        sub_c.reshape(-1)[:full] = sub[lo:lo + full]
        dom_c.reshape(-1)[:full] = dom[lo:lo + full]
        t = s_per - full
        if t:
            sub_c[-1, :t] = sub[lo + full:lo + s_per]
            dom_c[-1, :t] = dom[lo + full:lo + s_per]
        in_maps.append({
            "x": x_c, "sub_t": sub_c, "dom_t": dom_c, "cb": cb, "i33": i33,
        })
    return in_maps


def kernel(inputs, targets, domains, domain_counts):
    inputs = np.asarray(inputs, dtype=np.float32)
    targets_np = np.asarray(targets).reshape(-1)
    domains_np = np.asarray(domains).reshape(-1)
    counts = np.asarray(domain_counts, dtype=np.float32)

    n = inputs.shape[0]
    assert n % N_CORES == 0
    s_per = n // N_CORES                 # 125000
    u_pairs = 8
    n_iters = s_per // (u_pairs * W * P)  # 61
    tail = s_per - n_iters * u_pairs * W * P  # 72
    nb_rows = n_iters * u_pairs * W + (1 if tail else 0)

    nc = build_program(s_per, u_pairs, n_iters, tail, reps=1)
    in_maps = _host_prep(inputs, targets_np, domains_np, counts,
                         s_per, nb_rows)
    res = run_bass_kernel_spmd(nc, in_maps, list(range(N_CORES)))

    total = 0.0
    for r in res.results:
        total += r["acc"].astype(np.float64).sum()
    n_valid = int((targets_np != 255).sum())
    return np.float32(total / n_valid)

